# revision 1
# baseline (speedup 1.0000x reference)
"""ComplEx scoring kernel for 8 Trainium2 NeuronCores.

Math: score[b, e] = Re(<h_b * r_b, conj(ent_e)>) with h = ent_emb[triples[:,0]],
r = rel_emb[triples[:,1]].  Writing ans_b = concat(re_h*re_r - im_h*im_r,
re_h*im_r + im_h*re_r) (shape [B, 512]), the score is exactly
score = ans @ ent_emb.T  — one [1024, 512] x [512, 200000] GEMM.

Strategy (vocab/tensor parallel, per the entity axis):
  - host: tiny gather + complex multiply -> ans  (microseconds)
  - shard ent_emb rows 8 ways (25000/core, zero-padded to 25088 = 49*512),
    pre-transposed + bf16-cast on host so the device streams contiguous
    [K=512, E] tiles
  - each core: score_shard[1024, 25088] f32 = ansT.T @ entT via PE-array
    matmuls (bf16 in, fp32 PSUM accumulate), memory-bound streaming
  - host: concatenate the 8 column slabs, drop padding
"""

import numpy as np
import ml_dtypes

NCORES = 8
NUM_ENT = 200000
EMB = 512
B = 1024
SHARD = NUM_ENT // NCORES      # 25000 entities per core
NTILE = 512                    # matmul moving free dim == one PSUM bank
TPG = 7                        # 512-tiles per DMA group
GN = NTILE * TPG               # 3584 entities per group
NGROUPS = 7
SHARD_PAD = GN * NGROUPS       # 25088
KCH = EMB // 128               # 4 contraction chunks
MCH = B // 128                 # 8 batch chunks

_NC = None

# score values are ~1e-5 — subnormal in fp16.  Pre-scaling ans by 2**16 on
# the host puts the device-side scores in fp16's normal range, so the output
# can be stored/DMA'd as fp16 (half the write traffic); the host unscales.
OUT_SCALE = 2.0 ** 16


def _build_nc():
    import concourse.bacc as bacc
    import concourse.bass as bass
    import concourse.tile as tile
    from concourse import mybir

    ts, ds = bass.ts, bass.ds
    bf16 = mybir.dt.bfloat16
    f16 = mybir.dt.float16
    f32 = mybir.dt.float32

    nc = bacc.Bacc("TRN2", target_bir_lowering=False, debug=False)
    ansT = nc.dram_tensor("ansT", [EMB, B], bf16, kind="ExternalInput")
    entT = nc.dram_tensor("entT", [EMB, SHARD_PAD], bf16, kind="ExternalInput")
    score = nc.dram_tensor("score", [B, SHARD_PAD], f16, kind="ExternalOutput")

    with tile.TileContext(nc) as tc:
        with tc.tile_pool(name="const", bufs=1) as const_pool, \
             tc.tile_pool(name="entp", bufs=3 * KCH) as ent_pool, \
             tc.tile_pool(name="outp", bufs=3) as out_pool, \
             tc.tile_pool(name="ps", bufs=8, space="PSUM") as psum_pool:

            def load_group(g):
                # one tile per k-chunk so a matmul only waits for its own DMA
                tiles = []
                for k in range(KCH):
                    t = ent_pool.tile([128, GN], bf16, name="ent_sb")
                    nc.sync.dma_start(t[:], entT[ts(k, 128), ds(g * GN, GN)])
                    tiles.append(t)
                return tiles

            # startup: ansT first (small), then group 0 split into per-tile
            # DMAs in PE consume order so early matmuls start as soon as
            # their slice lands (all on the SP queue, in order)
            ansT_sb = const_pool.tile([128, KCH, B], bf16, name="ansT_sb")
            for k in range(KCH):
                nc.sync.dma_start(ansT_sb[:, k], ansT[ts(k, 128), :])
            # t-major issue order matches the first block's t-outer consume
            # order, so the first psum tile only waits for ~4 small DMAs
            ent_sb0 = [ent_pool.tile([128, GN], bf16, name="ent_sb")
                       for _ in range(KCH)]
            for tt in range(TPG):
                for k in range(KCH):
                    nc.sync.dma_start(ent_sb0[k][:, ts(tt, NTILE)],
                                      entT[ts(k, 128), ds(tt * NTILE, NTILE)])

            # gpsimd (Pool) cannot read PSUM on TRN2 — copyback on DVE + Act
            copy_engines = [nc.vector, nc.scalar]
            ci = 0
            ent_tiles = {0: ent_sb0}
            for g in range(NGROUPS):
                # prefetch next group's tiles ahead of this group's output DMAs
                # so the SP engine's in-order stream doesn't serialize them
                if g + 1 < NGROUPS:
                    ent_tiles[g + 1] = load_group(g + 1)
                ent_sb = ent_tiles.pop(g)
                for m in range(MCH):
                    pss = [psum_pool.tile([128, NTILE], f32, name="pst")
                           for _ in range(TPG)]
                    out_sb = out_pool.tile([128, GN], f16, name="out_sb")
                    # first block: t-outer so the PE starts on partial data;
                    # last block: t-outer so the drain overlaps the matmuls
                    t_outer = ((g == NGROUPS - 1) and (m == MCH - 1)) or \
                              (g == 0 and m == 0)

                    def copyback(t):
                        nonlocal ci
                        eng = copy_engines[ci % len(copy_engines)]
                        ci += 1
                        if eng is nc.scalar:
                            eng.copy(out_sb[:, ts(t, NTILE)], pss[t][:])
                        else:
                            eng.tensor_copy(out=out_sb[:, ts(t, NTILE)],
                                            in_=pss[t][:])

                    if not t_outer:
                        # k outer / tile inner: stationary weights switch once
                        # per TPG matmuls instead of every matmul
                        for k in range(KCH):
                            lhsT = ansT_sb[:, k, ts(m, 128)]
                            for t in range(TPG):
                                nc.tensor.matmul(
                                    pss[t][:],
                                    lhsT,
                                    ent_sb[k][:, ts(t, NTILE)],
                                    start=(k == 0),
                                    stop=(k == KCH - 1),
                                )
                        for t in range(TPG):
                            copyback(t)
                        # two half-width output DMAs so the drain starts as
                        # soon as the first copies land
                        h0 = 4 * NTILE
                        nc.sync.dma_start(score[ts(m, 128), ds(g * GN, h0)],
                                          out_sb[:, :h0])
                        nc.sync.dma_start(
                            score[ts(m, 128), ds(g * GN + h0, GN - h0)],
                            out_sb[:, h0:])
                    else:
                        # t-outer: each psum tile finishes its accumulation
                        # early; interleave copies + small DMAs so the drain
                        # (or warm-up) overlaps the remaining matmuls
                        for t in range(TPG):
                            for k in range(KCH):
                                nc.tensor.matmul(
                                    pss[t][:],
                                    ansT_sb[:, k, ts(m, 128)],
                                    ent_sb[k][:, ts(t, NTILE)],
                                    start=(k == 0),
                                    stop=(k == KCH - 1),
                                )
                            copyback(t)
                            if t % 2 == 1:
                                nc.sync.dma_start(
                                    score[ts(m, 128),
                                          ds(g * GN + (t - 1) * NTILE, 2 * NTILE)],
                                    out_sb[:, ds((t - 1) * NTILE, 2 * NTILE)])
                        nc.sync.dma_start(
                            score[ts(m, 128), ds(g * GN + (TPG - 1) * NTILE, NTILE)],
                            out_sb[:, ds((TPG - 1) * NTILE, NTILE)])
    nc.compile()
    return nc


def _get_nc():
    global _NC
    if _NC is None:
        _NC = _build_nc()
    return _NC


def _pmap(fn, n):
    from concurrent.futures import ThreadPoolExecutor
    with ThreadPoolExecutor(max_workers=n) as ex:
        list(ex.map(fn, range(n)))


def prepare_in_maps(triples, ent_emb, rel_emb):
    triples = np.asarray(triples)
    ent_emb = np.asarray(ent_emb, dtype=np.float32)
    rel_emb = np.asarray(rel_emb, dtype=np.float32)

    d = EMB // 2
    h = ent_emb[triples[:, 0].astype(np.int64)]
    r = rel_emb[triples[:, 1].astype(np.int64)]
    re_h, im_h = h[:, :d], h[:, d:]
    re_r, im_r = r[:, :d], r[:, d:]
    ans = np.empty((B, EMB), np.float32)
    ans[:, :d] = re_h * re_r - im_h * im_r
    ans[:, d:] = re_h * im_r + im_h * re_r
    ans *= np.float32(OUT_SCALE)
    ansT_bf = np.ascontiguousarray(ans.T).astype(ml_dtypes.bfloat16)

    ent_bf = np.empty(ent_emb.shape, dtype=ml_dtypes.bfloat16)
    shards = np.empty((NCORES, EMB, SHARD_PAD), dtype=ml_dtypes.bfloat16)

    def _cast(c):
        s = slice(c * SHARD, (c + 1) * SHARD)
        ent_bf[s] = ent_emb[s]

    def _shard(c):
        shards[c, :, :SHARD] = ent_bf[c * SHARD:(c + 1) * SHARD].T
        shards[c, :, SHARD:] = 0

    _pmap(_cast, NCORES)
    _pmap(_shard, NCORES)
    return [{"ansT": ansT_bf, "entT": shards[c]} for c in range(NCORES)]


def run_raw(in_maps, trace=False):
    from concourse import bass_utils
    return bass_utils.run_bass_kernel_spmd(
        _get_nc(), in_maps, core_ids=list(range(NCORES)), trace=trace
    )


def assemble(results):
    out = np.empty((B, NUM_ENT), np.float32)
    inv = np.float32(1.0 / OUT_SCALE)

    def _one(c):
        sh = results[c]["score"][:, :SHARD].astype(np.float32)
        sh *= inv
        out[:, c * SHARD:(c + 1) * SHARD] = sh

    _pmap(_one, NCORES)
    return out


def kernel(triples, ent_emb, rel_emb):
    in_maps = prepare_in_maps(triples, ent_emb, rel_emb)
    res = run_raw(in_maps)
    return assemble(res.results)

